# revision 1
# baseline (speedup 1.0000x reference)
"""Trainium2 Bass kernel for nn_Adapter (audio conv encoder + cross-attention).

Data-parallel over batch: 16 batches / 8 NeuronCores = 2 per core, no
collectives. All heavy matmuls run in bf16 (1 cycle/row on the PE array);
PSUM accumulation is fp32 throughout, output is fp32.
"""
import sys
sys.path.insert(0, "/opt/trn_rl_repo")

import numpy as np
import ml_dtypes

import concourse.bass as bass
import concourse.mybir as mybir
import concourse.tile as tile
from concourse.bass_utils import run_bass_kernel_spmd

F32 = mybir.dt.float32
BF16 = mybir.dt.bfloat16
AF = mybir.ActivationFunctionType
BF = ml_dtypes.bfloat16

NCORES = 8
B, N, CTX = 16, 4096, 768
BP = B // NCORES            # batches per core
H, D, INNER = 8, 40, 320    # heads, dim_head, inner
AUD = 1024                  # audio feature length
KS, PAD = 17, 8
EPS = 1e-5
SCALE = D ** -0.5
TCH = 512                   # token chunk
NCH = N // TCH              # chunks per batch

# pair -> sim matmul plan: (kp_tile_index, qt_chunk)
SIM_PLAN = [
    [(0, 0)],           # pair 0 (h0,h1): KP01 x qt_ch0
    [(1, 0), (2, 1)],   # pair 1 (h2,h3): KP23a x ch0 + KP23b x ch1
    [(3, 1)],           # pair 2 (h4,h5): KP45 x ch1
    [(4, 1), (5, 2)],   # pair 3 (h6,h7): KP67a x ch1 + KP67b x ch2
]
KP_DEF = [(0, 0), (0, 1), (1, 1), (1, 2), (1, 3), (2, 3)]  # tile -> (chunk, pair)
VM_DEF = [(0, 0), (0, 1), (1, 1), (1, 2), (1, 3), (2, 3)]  # v tiles  (chunk, pair)
AT_V = {0: [0, 1], 1: [1, 2, 3], 2: [3]}                    # chunk -> pairs with v
ME = [128, 128, 64]                                         # e-chunk sizes


def _head_of(e):
    return e // D


def _build_host_consts(inputs):
    c = {}
    w1, b1 = inputs["w1"], inputs["b1"]
    w2, b2 = inputs["w2"], inputs["b2"]
    w3, b3 = inputs["w3"], inputs["b3"]
    c["w1t"] = np.ascontiguousarray(w1[:, 0, :].T).astype(BF)             # [17, 64]

    def pack_pairs(w):  # [64co, 64ci, 17] -> [128, 9, 64co]
        wp = np.zeros((128, 9, 64), np.float32)
        wt = w.transpose(1, 2, 0)  # [ci, k, co]
        for q in range(9):
            wp[0:64, q, :] = wt[:, 2 * q, :]
            if 2 * q + 1 < KS:
                wp[64:128, q, :] = wt[:, 2 * q + 1, :]
        return wp.astype(BF)

    c["w2p"] = pack_pairs(w2)
    c["w3p"] = pack_pairs(w3)
    c["b1c"] = np.asarray(b1).reshape(64, 1).astype(np.float32)
    c["b2c"] = np.asarray(b2).reshape(64, 1).astype(np.float32)
    c["b3c"] = np.asarray(b3).reshape(64, 1).astype(np.float32)
    c["ln_w"] = np.asarray(inputs["ln_w"]).astype(np.float32)
    c["ln_b"] = np.asarray(inputs["ln_b"]).astype(np.float32)

    wqt = np.zeros((CTX, 384), np.float32)
    wqt[:, :INNER] = np.asarray(inputs["wq"]).T
    c["wqt"] = wqt.astype(BF)
    wkt = np.zeros((AUD, 384), np.float32)
    wkt[:, :INNER] = np.asarray(inputs["wk"]).T
    c["wkt"] = wkt.astype(BF)
    c["wvt"] = np.ascontiguousarray(np.asarray(inputs["wv"]).T).astype(BF)
    wout = np.zeros((384, CTX), np.float32)
    wout[:INNER] = np.asarray(inputs["w_out"]).T
    wout[INNER] = np.asarray(inputs["b_out"])
    c["woutA"] = wout.astype(BF)

    km = np.zeros((128, 6, 128), np.float32)
    for t, (n, p) in enumerate(KP_DEF):
        for r in range(ME[n]):
            h = _head_of(128 * n + r)
            if h == 2 * p:
                km[r, t, 0:64] = 1.0
            elif h == 2 * p + 1:
                km[r, t, 64:128] = 1.0
    c["kmask"] = km.astype(BF)

    vm = np.zeros((128, 6, 128), np.float32)
    for t, (n, p) in enumerate(VM_DEF):
        for col in range(ME[n]):
            h = _head_of(128 * n + col)
            if h == 2 * p:
                vm[0:64, t, col] = 1.0
            elif h == 2 * p + 1:
                vm[64:128, t, col] = 1.0
    c["vmask"] = vm.astype(BF)

    e8 = np.zeros((72, 3, 128), np.float32)
    for n in range(3):
        for r in range(ME[n]):
            e8[64 + _head_of(128 * n + r), n, r] = 1.0
    c["exp8"] = e8.astype(BF)

    c["ident"] = np.eye(64, dtype=np.float32)
    return c


def _build_graph():
    nc = bass.Bass()
    P = {}

    def inp(name, shape, dt):
        P[name] = nc.declare_dram_parameter(name, list(shape), dt, isOutput=False)

    inp("ctx16", (BP, CTX, N), BF16)
    inp("a_im", (BP, KS, AUD), BF16)
    inp("w1t", (KS, 64), BF16)
    inp("w2p", (128, 9, 64), BF16)
    inp("w3p", (128, 9, 64), BF16)
    inp("b1c", (64, 1), F32)
    inp("b2c", (64, 1), F32)
    inp("b3c", (64, 1), F32)
    inp("ln_w", (64, AUD), F32)
    inp("ln_b", (64, AUD), F32)
    inp("wqt", (CTX, 384), BF16)
    inp("wkt", (AUD, 384), BF16)
    inp("wvt", (AUD, INNER), BF16)
    inp("woutA", (384, CTX), BF16)
    inp("kmask", (128, 6, 128), BF16)
    inp("vmask", (128, 6, 128), BF16)
    inp("exp8", (72, 3, 128), BF16)
    inp("ident", (64, 64), F32)
    out_e = nc.declare_dram_parameter("out", [BP, N, CTX], F32, isOutput=True)

    with tile.TileContext(nc) as tc:
        cp = tc.alloc_tile_pool(name="const", bufs=1)
        pp = tc.alloc_tile_pool(name="persist", bufs=1)
        cinp = tc.alloc_tile_pool(name="cinp", bufs=4)
        esp = tc.alloc_tile_pool(name="esp", bufs=6)
        qtp = tc.alloc_tile_pool(name="qtp", bufs=4)
        mp = tc.alloc_tile_pool(name="mp", bufs=2)
        ofp = tc.alloc_tile_pool(name="ofp", bufs=4)
        ap = tc.alloc_tile_pool(name="audio", bufs=1)
        aps = tc.alloc_tile_pool(name="aps", bufs=2, space="PSUM")

        # ---- constants ----
        def cload(name, shape, dt, ap_src=None):
            t = cp.tile(list(shape), dt, tag=name)
            nc.sync.dma_start(t[:], ap_src if ap_src is not None else P[name][:])
            return t

        wqt = cload("wqt", (128, 6, 384), BF16,
                    P["wqt"][:].rearrange("(n p) e -> p n e", p=128))
        cin_pre = {}
        for c0_ in range(4):
            t = cinp.tile([128, 6, TCH], BF16, tag="cin")
            nc.gpsimd.dma_start(
                t[:], P["ctx16"][0].rearrange("(n p) t -> p n t", p=128)
                [:, :, TCH * c0_:TCH * c0_ + TCH])
            cin_pre[(0, c0_)] = t

        w1t = cload("w1t", (KS, 64), BF16)
        w2p = cload("w2p", (128, 9, 64), BF16)
        w3p = cload("w3p", (128, 9, 64), BF16)
        b1c = cload("b1c", (64, 1), F32)
        b2c = cload("b2c", (64, 1), F32)
        b3c = cload("b3c", (64, 1), F32)
        lnw = cload("ln_w", (64, AUD), F32)
        lnb = cload("ln_b", (64, AUD), F32)
        wkt = cload("wkt", (128, 8, 384), BF16,
                    P["wkt"][:].rearrange("(n p) e -> p n e", p=128))
        wvt = cload("wvt", (128, 8, INNER), BF16,
                    P["wvt"][:].rearrange("(n p) e -> p n e", p=128))
        woutA = cload("woutA", (128, 3, CTX), BF16,
                      P["woutA"][:].rearrange("(n p) c -> p n c", p=128))
        kmask = cload("kmask", (128, 6, 128), BF16)
        vmask = cload("vmask", (128, 6, 128), BF16)
        exp8 = cload("exp8", (72, 3, 128), BF16)
        ident = cload("ident", (64, 64), F32)

        ones64 = cp.tile([64, 64], BF16, tag="ones64")
        nc.vector.memset(ones64[:], 1.0)

        # denominator-only VP tiles (batch-independent)
        vpd = []
        for p in range(3):
            t = cp.tile([128, 72], BF16, tag=f"vpd{p}")
            nc.gpsimd.memset(t[:], 0.0)
            nc.gpsimd.memset(t[0:64, 64 + 2 * p: 65 + 2 * p], 1.0)
            nc.gpsimd.memset(t[64:128, 65 + 2 * p: 66 + 2 * p], 1.0)
            vpd.append(t)

        def emit_q(cin, psum_pool, psum_tag):
            qt = qtp.tile([128, 3, TCH], BF16, tag="qt")
            for m in range(3):
                qp = psum_pool.tile([128, TCH], F32, tag=psum_tag)
                for n6 in range(6):
                    nc.tensor.matmul(qp[:], wqt[:, n6, 128 * m:128 * m + 128],
                                     cin[:, n6, :], start=(n6 == 0), stop=(n6 == 5))
                nc.vector.tensor_copy(qt[:, m, :], qp[:])
            return qt

        qt_pre = {}
        qt_pre[(0, 0)] = emit_q(cin_pre[(0, 0)], aps, "qpre")

        # ---- audio encoder phases (b0 pre-main; b1 injected into main loop) ----
        kp_all, vp_all = [None, None], [None, None]
        a_sbs, xb2s, x2s, statss, xb3s, x_sbs, xts = {}, {}, {}, {}, {}, {}, {}
        PADB = AUD + 2 * PAD

        for b in range(BP):
            a_sb = ap.tile([KS, AUD], BF16, tag=f"a_im{b}")
            nc.sync.dma_start(a_sb[:], P["a_im"][b])
            a_sbs[b] = a_sb

        def ph_conv1(b, psp, cvtag):
            xb2 = ap.tile([128, PADB], BF16, tag=f"xb2{b}")
            nc.gpsimd.memset(xb2[0:64, 0:PAD], 0.0)
            nc.gpsimd.memset(xb2[0:64, AUD + PAD:PADB], 0.0)
            nc.gpsimd.memset(xb2[64:128, PADB - 1:PADB], 0.0)
            for cc in range(2):
                cv1 = psp.tile([64, 512], F32, tag=cvtag)
                nc.tensor.matmul(cv1[:], w1t[:], a_sbs[b][:, 512 * cc:512 * cc + 512],
                                 start=True, stop=True)
                nc.scalar.activation(xb2[0:64, PAD + 512 * cc: PAD + 512 * cc + 512],
                                     cv1[:], AF.Gelu, bias=b1c[:])
            nc.sync.dma_start(xb2[64:128, 0:PADB - 1], xb2[0:64, 1:PADB])
            xb2s[b] = xb2

        def ph_conv2(b, psp, cvtag):
            x2 = ap.tile([64, AUD], F32, tag=f"x2{b}")
            stats = ap.tile([64, 4], F32, tag=f"stats{b}")
            sq_scr = ap.tile([64, 512], F32, tag=f"sq{b}")
            for cc in range(2):
                cv2 = psp.tile([64, 512], F32, tag=cvtag)
                for q in range(9):
                    nc.tensor.matmul(cv2[:], w2p[:, q, :],
                                     xb2s[b][:, 2 * q + 512 * cc: 2 * q + 512 * cc + 512],
                                     start=(q == 0), stop=(q == 8))
                nc.vector.tensor_scalar(
                    out=x2[:, 512 * cc:512 * cc + 512], in0=cv2[:],
                    scalar1=b2c[:], scalar2=0.0, op0=mybir.AluOpType.add,
                    op1=mybir.AluOpType.add, accum_out=stats[:, cc:cc + 1])
                nc.vector.tensor_mul(sq_scr[:], x2[:, 512 * cc:512 * cc + 512],
                                     x2[:, 512 * cc:512 * cc + 512])
                nc.vector.reduce_sum(stats[:, 2 + cc:3 + cc], sq_scr[:],
                                     axis=mybir.AxisListType.X)
            x2s[b] = x2
            statss[b] = stats

        def ph_ln(b, psp, cvtag):
            stats = statss[b]
            x2 = x2s[b]
            tot16 = ap.tile([64, 2], BF16, tag=f"tot16{b}")
            nc.vector.tensor_add(tot16[:, 0:1], stats[:, 0:1], stats[:, 1:2])
            nc.vector.tensor_add(tot16[:, 1:2], stats[:, 2:3], stats[:, 3:4])
            totp = psp.tile([64, 64], F32, tag=cvtag)
            nc.tensor.matmul(totp[:, 0:2], ones64[:], tot16[:], start=True, stop=True)

            mu = ap.tile([64, 1], F32, tag=f"mu{b}")
            msq = ap.tile([64, 1], F32, tag=f"msq{b}")
            var = ap.tile([64, 1], F32, tag=f"var{b}")
            sd = ap.tile([64, 1], F32, tag=f"sd{b}")
            rstd = ap.tile([64, 1], F32, tag=f"rstd{b}")
            nmr = ap.tile([64, 1], F32, tag=f"nmr{b}")
            inv_n = 1.0 / (64 * AUD)
            nc.vector.tensor_scalar_mul(mu[:], totp[:, 0:1], inv_n)
            nc.vector.tensor_scalar_mul(msq[:], totp[:, 1:2], inv_n)
            nc.vector.tensor_mul(var[:], mu[:], mu[:])
            nc.vector.tensor_sub(var[:], msq[:], var[:])
            nc.vector.tensor_scalar_add(var[:], var[:], EPS)
            nc.scalar.activation(sd[:], var[:], AF.Ln)
            nc.scalar.activation(rstd[:], sd[:], AF.Exp, scale=-0.5)
            nc.vector.tensor_mul(nmr[:], mu[:], rstd[:])
            nc.vector.tensor_scalar_mul(nmr[:], nmr[:], -1.0)

            t1 = ap.tile([64, AUD], F32, tag=f"t1{b}")
            t2 = ap.tile([64, AUD], F32, tag=f"t2{b}")
            xb3 = ap.tile([128, PADB], BF16, tag=f"xb3{b}")
            nc.gpsimd.memset(xb3[0:64, 0:PAD], 0.0)
            nc.gpsimd.memset(xb3[0:64, AUD + PAD:PADB], 0.0)
            nc.gpsimd.memset(xb3[64:128, PADB - 1:PADB], 0.0)
            nc.vector.tensor_scalar(out=t1[:], in0=x2[:], scalar1=rstd[:],
                                    scalar2=nmr[:], op0=mybir.AluOpType.mult,
                                    op1=mybir.AluOpType.add)
            nc.vector.tensor_mul(t2[:], t1[:], lnw[:])
            nc.vector.tensor_add(xb3[0:64, PAD:PAD + AUD], t2[:], lnb[:])
            nc.sync.dma_start(xb3[64:128, 0:PADB - 1], xb3[0:64, 1:PADB])
            xb3s[b] = xb3

        def ph_conv3(b, psp, cvtag):
            x_sb = ap.tile([64, AUD], F32, tag=f"x_sb{b}")
            for cc in range(2):
                cv3 = psp.tile([64, 512], F32, tag=cvtag)
                for q in range(9):
                    nc.tensor.matmul(cv3[:], w3p[:, q, :],
                                     xb3s[b][:, 2 * q + 512 * cc: 2 * q + 512 * cc + 512],
                                     start=(q == 0), stop=(q == 8))
                nc.vector.tensor_scalar(
                    out=x_sb[:, 512 * cc:512 * cc + 512], in0=cv3[:],
                    scalar1=b3c[:], scalar2=0.0, op0=mybir.AluOpType.add,
                    op1=mybir.AluOpType.add)
            x_sbs[b] = x_sb

        def ph_xt(b, psp, cvtag):
            xt = pp.tile([128, 8, 64], BF16, tag=f"xt{b}")
            for f in range(8):
                pt = psp.tile([128, 64], F32, tag=cvtag)
                nc.tensor.transpose(pt[:], x_sbs[b][:, 128 * f:128 * f + 128], ident[:])
                nc.scalar.activation(xt[:, f, :], pt[:], AF.Copy)
            xts[b] = xt

        def ph_ktv(b, psp, cvtag):
            xt = xts[b]
            kt = pp.tile([128, 3, 64], BF16, tag=f"kt{b}")
            for m in range(3):
                ktp = psp.tile([128, 64], F32, tag=cvtag)
                for aj in range(8):
                    nc.tensor.matmul(ktp[:], wkt[:, aj, 128 * m:128 * m + 128],
                                     xt[:, aj, :], start=(aj == 0), stop=(aj == 7))
                nc.scalar.activation(kt[:, m, :], ktp[:], AF.Copy)

            v2p = psp.tile([128, INNER], F32, tag=cvtag)
            for half in range(2):
                for aj in range(8):
                    nc.tensor.matmul(v2p[64 * half:64 * half + 64, :],
                                     xt[:, aj, :], wvt[:, aj, :],
                                     start=(aj == 0), stop=(aj == 7))
            v2 = pp.tile([128, INNER], BF16, tag=f"v2{b}")
            nc.scalar.activation(v2[:], v2p[:], AF.Copy)

            kps = []
            for t, (n, p) in enumerate(KP_DEF):
                kpt = pp.tile([128, 128], BF16, tag=f"kp{t}_{b}")
                nc.vector.tensor_mul(
                    kpt[:].rearrange("p (a b) -> p a b", a=2),
                    kt[:, n:n + 1, :].broadcast_to([128, 2, 64]),
                    kmask[:, t, :].rearrange("p (a b) -> p a b", a=2))
                kps.append(kpt)
            kp_all[b] = kps

            vps = {}
            for t, (n, p) in enumerate(VM_DEF):
                w = 72 if n == 2 else ME[n]
                vpt = pp.tile([128, w], BF16, tag=f"vp{t}_{b}")
                nc.vector.tensor_mul(vpt[:, 0:ME[n]],
                                     v2[:, 128 * n:128 * n + ME[n]],
                                     vmask[:, t, 0:ME[n]])
                if n == 2:
                    nc.gpsimd.memset(vpt[:, 64:72], 0.0)
                    nc.gpsimd.memset(vpt[0:64, 64 + 2 * p:65 + 2 * p], 1.0)
                    nc.gpsimd.memset(vpt[64:128, 65 + 2 * p:66 + 2 * p], 1.0)
                vps[(n, p)] = vpt
            vp_all[b] = vps

        # batch 0 audio now, with q prefetches as PE filler
        ph_conv1(0, aps, "cv")
        ph_conv1(1, aps, "cv")
        qt_pre[(0, 1)] = emit_q(cin_pre[(0, 1)], aps, "qpre")
        ph_conv2(0, aps, "cv")
        qt_pre[(0, 2)] = emit_q(cin_pre[(0, 2)], aps, "qpre")
        ph_ln(0, aps, "cv")
        ph_conv3(0, aps, "cv")
        qt_pre[(0, 3)] = emit_q(cin_pre[(0, 3)], aps, "qpre")
        ph_xt(0, aps, "cv")
        ph_ktv(0, aps, "cv")

        aps.release()

        # ---- main attention loop ----
        mps = tc.alloc_tile_pool(name="mps", bufs=2, space="PSUM")

        pending_out = None

        def emit_out(job):
            ob, oc, oat = job
            for tt in range(4):
                of = ofp.tile([128, CTX], F32, tag="of")
                for ci, (c0, cw) in enumerate(((0, 384), (384, 384))):
                    op = mps.tile([128, 512], F32, tag="ob")
                    for n in range(3):
                        rows = 65 if n == 2 else 128
                        nc.tensor.matmul(
                            op[:, 0:cw],
                            oat[0:rows, n, 128 * tt:128 * tt + 128],
                            woutA[0:rows, n, c0:c0 + cw],
                            start=(n == 0), stop=(n == 2))
                    if ci == 0:
                        nc.scalar.activation(of[:, c0:c0 + cw], op[:, 0:cw],
                                             AF.Copy)
                    else:
                        nc.vector.tensor_copy(of[:, c0:c0 + cw], op[:, 0:cw])
                nc.sync.dma_start(
                    out_e[ob, TCH * oc + 128 * tt: TCH * oc + 128 * tt + 128, :],
                    of[:])

        chunks = [(bb, cc2) for bb in range(BP) for cc2 in range(NCH)]
        ctx_aps = [P["ctx16"][bb].rearrange("(n p) t -> p n t", p=128)
                   for bb in range(BP)]
        cins = dict(cin_pre)
        qts = {i2: qt_pre[(0, i2)] for i2 in range(4)}

        b1_phases = {0: lambda: ph_conv2(1, mps, "qp"),
                     2: lambda: ph_ln(1, mps, "qp"),
                     4: lambda: ph_conv3(1, mps, "qp"),
                     5: lambda: ph_xt(1, mps, "qp"),
                     6: lambda: ph_ktv(1, mps, "qp")}

        for i, (b, c) in enumerate(chunks):
            kps = kp_all[b]
            vps = vp_all[b]
            for la in (3, 4):
                if i + la < len(chunks) and chunks[i + la] not in cins:
                    b3, c3 = chunks[i + la]
                    t = cinp.tile([128, 6, TCH], BF16, tag="cin")
                    nc.gpsimd.dma_start(t[:],
                                        ctx_aps[b3][:, :, TCH * c3:TCH * c3 + TCH])
                    cins[chunks[i + la]] = t

            qt = qts.pop(i)

            es = []
            for p in range(4):
                sp = mps.tile([128, TCH], F32, tag="sp")
                plan = SIM_PLAN[p]
                for ii, (kpi, qch) in enumerate(plan):
                    nc.tensor.matmul(sp[:], kps[kpi][:], qt[:, qch, :],
                                     start=(ii == 0), stop=(ii == len(plan) - 1))
                e = esp.tile([128, TCH], BF16, tag="es")
                nc.scalar.activation(e[:], sp[:], AF.Exp, scale=SCALE)
                es.append(e)

            # q for chunk i+2 fills the PE while exp runs on ACT
            if i + 2 < len(chunks) and (i + 2) not in qts:
                qts[i + 2] = emit_q(cins.pop(chunks[i + 2]), mps, "qp")

            at_sb = mp.tile([128, 3, TCH], BF16, tag="at_sb")
            nc.gpsimd.memset(at_sb[64:65, 2, :], 1.0)

            at2 = mps.tile([128, TCH], F32, tag="at")
            n2_ops = [(vpd[0], 0), (vpd[1], 1), (vpd[2], 2), (vps[(2, 3)], 3)]
            for ii, (vpt, p) in enumerate(n2_ops):
                nc.tensor.matmul(at2[0:72, :], vpt[:], es[p][:],
                                 start=(ii == 0), stop=(ii == 3))
            lnd = mp.tile([72, TCH], F32, tag="lnd")
            rec16 = mp.tile([72, TCH], BF16, tag="rec16")
            nc.scalar.activation(lnd[64:72, :], at2[64:72, :], AF.Ln)
            nc.scalar.activation(rec16[64:72, :], lnd[64:72, :], AF.Exp,
                                 scale=-1.0)

            def normalize(n, at_ps):
                brp = mps.tile([128, TCH], F32, tag="ob")
                nc.tensor.matmul(brp[0:ME[n], :], exp8[64:72, n, 0:ME[n]],
                                 rec16[64:72, :], start=True, stop=True)
                brs = mp.tile([128, TCH], F32, tag="brs")
                nc.vector.tensor_copy(brs[0:ME[n], :], brp[0:ME[n], :])
                nc.vector.tensor_mul(at_sb[0:ME[n], n, :],
                                     at_ps[0:ME[n], :], brs[0:ME[n], :])

            for n in (0, 1):
                a = mps.tile([128, TCH], F32, tag="at")
                prs = AT_V[n]
                for ii, p in enumerate(prs):
                    nc.tensor.matmul(a[0:ME[n], :], vps[(n, p)][:, 0:ME[n]],
                                     es[p][:], start=(ii == 0),
                                     stop=(ii == len(prs) - 1))
                normalize(n, a)
            normalize(2, at2)

            if pending_out is not None:
                emit_out(pending_out)
            if i in b1_phases:
                b1_phases[i]()
            pending_out = (b, c, at_sb)

        emit_out(pending_out)

        mps.release()
        ap.release()
        ofp.release()
        mp.release()
        qtp.release()
        esp.release()
        cinp.release()
        pp.release()
        cp.release()

    split_waits(nc)
    return nc


def split_waits(nc, max_waits=1):
    """neuronxcc walrus accepts at most one attached sync wait per
    instruction; hoist extras onto standalone event-semaphore waits."""
    n_new = 0
    for f in nc.m.functions:
        for blk in f.blocks:
            new = []
            changed = False
            for inst in blk.instructions:
                si = inst.sync_info
                ow = list(si.on_wait) if (si is not None and si.on_wait) else []
                if len(ow) > max_waits:
                    for w in ow[:-max_waits]:
                        ev = mybir.InstEventSemaphore(
                            name=f"I-waitsplit-{n_new}", ins=[], outs=[])
                        ev.engine = inst.engine
                        ev.sync_info = mybir.SyncInfo(on_wait=[w], on_update=[])
                        nc.register_instruction(ev)
                        new.append(ev)
                        n_new += 1
                    inst.sync_info = mybir.SyncInfo(
                        on_wait=ow[-max_waits:], on_update=list(si.on_update))
                    changed = True
                new.append(inst)
            if changed:
                blk.instructions = new


_GRAPH = None


def kernel(**inputs):
    global _GRAPH
    if _GRAPH is None:
        _GRAPH = _build_graph()
    nc = _GRAPH

    inputs = {k: np.asarray(v, dtype=np.float32) for k, v in inputs.items()}
    consts = _build_host_consts(inputs)
    ctx = np.asarray(inputs["context"])           # [16, 4096, 768] f32
    audio = np.asarray(inputs["audio_context"])   # [16, 1, 1024] f32

    ctx16 = np.ascontiguousarray(ctx.transpose(0, 2, 1)).astype(BF)
    apad = np.zeros((B, AUD + 2 * PAD), np.float32)
    apad[:, PAD:PAD + AUD] = audio[:, 0, :]
    a_im = np.empty((B, KS, AUD), np.float32)
    for k in range(KS):
        a_im[:, k, :] = apad[:, k:k + AUD]
    a_im = a_im.astype(BF)

    in_maps = []
    for core in range(NCORES):
        m = dict(consts)
        s = slice(core * BP, (core + 1) * BP)
        m["ctx16"] = ctx16[s]
        m["a_im"] = a_im[s]
        in_maps.append(m)

    res = run_bass_kernel_spmd(nc, in_maps, list(range(NCORES)))
    out = np.concatenate([res.results[i]["out"] for i in range(NCORES)], axis=0)
    return out.astype(np.float32)



# revision 2
# speedup vs baseline: 1.2281x; 1.2281x over previous
"""Trainium2 Bass kernel for nn_Adapter (audio conv encoder + cross-attention), v4.

Baseline pipeline + surgical wins:
  - permuted e'-layout [c0: pair0+pair1a | c1: pair2+pair1b | c2: pair3]:
    sim 6->5 MMs, AV 9->8 MMs per chunk; denominators at psum rows 96:104
  - audio convs col-tiled (two concurrent MMs on array col-halves)
  - bf16 output; audio-first const loads; xb2/xb3 assembly DMAs on scalar queue
"""
import sys
sys.path.insert(0, "/opt/trn_rl_repo")

import numpy as np
import ml_dtypes

import concourse.bass as bass
import concourse.mybir as mybir
import concourse.tile as tile
from concourse.bass_utils import run_bass_kernel_spmd

F32 = mybir.dt.float32
BF16 = mybir.dt.bfloat16
AF = mybir.ActivationFunctionType
BF = ml_dtypes.bfloat16

NCORES = 8
B, N, CTX = 16, 4096, 768
BP = B // NCORES
H, D, INNER = 8, 40, 320
AUD = 1024
KS, PAD = 17, 8
EPS = 1e-5
SCALE = D ** -0.5
TCH = 512
NCH = N // TCH
PADB = AUD + 2 * PAD

# e' permutation: slot -> old e (-1 = pad); bias at slot 352 (chunk2 row 96)
def _perm_new2old():
    p = [-1] * 384
    for s in range(128):
        p[s] = s                    # c0: pair0 (0:80) + pair1 dims 0:48
    for i in range(80):
        p[128 + i] = 160 + i        # c1 rows 0:80: pair2
    for i in range(32):
        p[208 + i] = 128 + i        # c1 rows 80:112: pair1 dims 48:80
    for i in range(80):
        p[256 + i] = 240 + i        # c2 rows 0:80: pair3
    return p


PN2O = _perm_new2old()
RN = [128, 112, 97]      # out-proj contraction rows (96=bias)
ME = [128, 112, 80]      # value rows per e'-chunk (normalize extent)


def _head_of_slot(n, r):
    o = PN2O[128 * n + r]
    return -1 if o < 0 else o // D


def _build_host_consts(inputs):
    c = {}
    w1, b1 = inputs["w1"], inputs["b1"]
    w2, b2 = inputs["w2"], inputs["b2"]
    w3, b3 = inputs["w3"], inputs["b3"]
    c["w1t"] = np.ascontiguousarray(w1[:, 0, :].T).astype(BF)

    def pack_pairs(w):
        wp = np.zeros((128, 9, 64), np.float32)
        wt = w.transpose(1, 2, 0)
        for q in range(9):
            wp[0:64, q, :] = wt[:, 2 * q, :]
            if 2 * q + 1 < KS:
                wp[64:128, q, :] = wt[:, 2 * q + 1, :]
        return wp.astype(BF)

    c["w2p"] = pack_pairs(w2)
    c["w3p"] = pack_pairs(w3)

    def dup2(v):
        return np.tile(np.asarray(v), 2).reshape(128, 1).astype(np.float32)

    c["b1c2"] = dup2(b1)
    c["b2c2"] = dup2(b2)
    c["b3c2"] = dup2(b3)
    lnw = np.asarray(inputs["ln_w"]).astype(np.float32)
    lnb = np.asarray(inputs["ln_b"]).astype(np.float32)
    c["lnw2"] = np.concatenate([lnw[:, 0:512], lnw[:, 512:1024]], 0)
    c["lnb2"] = np.concatenate([lnb[:, 0:512], lnb[:, 512:1024]], 0)

    wq = np.asarray(inputs["wq"])
    wk = np.asarray(inputs["wk"])
    wv = np.asarray(inputs["wv"])
    wout = np.asarray(inputs["w_out"])
    bout = np.asarray(inputs["b_out"])
    wqt = np.zeros((CTX, 384), np.float32)
    wkt = np.zeros((AUD, 384), np.float32)
    wvt = np.zeros((AUD, 384), np.float32)
    wA = np.zeros((384, CTX), np.float32)
    for s, o in enumerate(PN2O):
        if o >= 0:
            wqt[:, s] = wq[o, :]
            wkt[:, s] = wk[o, :]
            wvt[:, s] = wv[o, :]
            wA[s] = wout[:, o]
    wA[352] = bout
    c["wqt"] = wqt.astype(BF)
    c["wkt"] = wkt.astype(BF)
    c["wvt"] = wvt.astype(BF)
    c["woutA"] = wA.astype(BF)

    km_lo = np.zeros((80, 128), np.float32)
    km_lo[0:40, 0:64] = 1.0
    km_lo[40:80, 64:128] = 1.0
    c["kmask_lo"] = km_lo.astype(BF)
    km1a = np.zeros((128, 128), np.float32)
    km1a[80:120, 0:64] = 1.0
    km1a[120:128, 64:128] = 1.0
    c["kmask1a"] = km1a.astype(BF)
    km1b = np.zeros((128, 128), np.float32)
    km1b[80:112, 64:128] = 1.0
    c["kmask1b"] = km1b.astype(BF)

    vm = np.zeros((128, 80), np.float32)
    vm[0:64, 0:40] = 1.0
    vm[64:128, 40:80] = 1.0
    c["vmask_v"] = vm.astype(BF)

    # exp8': [104, 3, 128] head->row broadcast selector (rows 96:104 used)
    e8 = np.zeros((104, 3, 128), np.float32)
    for n in range(3):
        for r in range(ME[n]):
            h = _head_of_slot(n, r)
            if h >= 0:
                e8[96 + h, n, r] = 1.0
    c["exp8"] = e8.astype(BF)

    idA = np.zeros((128, 64), np.float32)
    idA[0:64] = np.eye(64)
    idA[64:128] = np.eye(64)
    c["identA"] = idA
    return c


def _build_graph():
    nc = bass.Bass()
    P = {}

    def inp(name, shape, dt):
        P[name] = nc.declare_dram_parameter(name, list(shape), dt, isOutput=False)

    inp("ctx16", (BP, CTX, N), BF16)
    inp("a_im", (BP, KS, AUD), BF16)
    inp("w1t", (KS, 64), BF16)
    inp("w2p", (128, 9, 64), BF16)
    inp("w3p", (128, 9, 64), BF16)
    inp("b1c2", (128, 1), F32)
    inp("b2c2", (128, 1), F32)
    inp("b3c2", (128, 1), F32)
    inp("lnw2", (128, 512), F32)
    inp("lnb2", (128, 512), F32)
    inp("wqt", (CTX, 384), BF16)
    inp("wkt", (AUD, 384), BF16)
    inp("wvt", (AUD, 384), BF16)
    inp("woutA", (384, CTX), BF16)
    inp("kmask_lo", (80, 128), BF16)
    inp("kmask1a", (128, 128), BF16)
    inp("kmask1b", (128, 128), BF16)
    inp("vmask_v", (128, 80), BF16)
    inp("exp8", (104, 3, 128), BF16)
    inp("identA", (128, 64), F32)
    out_e = nc.declare_dram_parameter("out", [BP, N, CTX], BF16, isOutput=True)

    with tile.TileContext(nc) as tc:
        cp = tc.alloc_tile_pool(name="const", bufs=1)
        pp = tc.alloc_tile_pool(name="persist", bufs=1)
        cinp = tc.alloc_tile_pool(name="cinp", bufs=4)
        esp = tc.alloc_tile_pool(name="esp", bufs=6)
        qtp = tc.alloc_tile_pool(name="qtp", bufs=4)
        mp = tc.alloc_tile_pool(name="mp", bufs=2)
        ofp = tc.alloc_tile_pool(name="ofp", bufs=4)
        ap = tc.alloc_tile_pool(name="audio", bufs=1)
        aps = tc.alloc_tile_pool(name="aps", bufs=2, space="PSUM")

        def cload(name, shape, dt, ap_src=None):
            t = cp.tile(list(shape), dt, tag=name)
            nc.sync.dma_start(t[:], ap_src if ap_src is not None else P[name][:])
            return t

        # ---- audio-path constants + inputs first ----
        w1t = cload("w1t", (KS, 64), BF16)
        a_sbs = {}
        for b in range(BP):
            a_sb = ap.tile([KS, AUD], BF16, tag=f"a_im{b}")
            nc.sync.dma_start(a_sb[:], P["a_im"][b])
            a_sbs[b] = a_sb
        b1c2 = cload("b1c2", (128, 1), F32)
        w2p = cload("w2p", (128, 9, 64), BF16)

        ones128 = cp.tile([128, 128], BF16, tag="ones128")
        nc.vector.memset(ones128[:], 1.0)

        # ---- audio encoder phases (dual-row [128, 512] layout) ----
        xb2s, x2bs, statss, xb3s, x_sbs, xts = {}, {}, {}, {}, {}, {}
        kp_all, vp_all = [None, None], [None, None]

        def asm_dual(dst, g):
            nc.gpsimd.memset(dst[0:64, 0:PAD], 0.0)
            nc.gpsimd.memset(dst[0:64, AUD + PAD:PADB], 0.0)
            nc.gpsimd.memset(dst[64:128, 0:PAD - 1], 0.0)
            nc.gpsimd.memset(dst[64:128, PAD + AUD - 1:PADB], 0.0)
            nc.scalar.dma_start(dst[0:64, PAD:PAD + 512], g[0:64, :])
            nc.scalar.dma_start(dst[0:64, PAD + 512:PAD + 1024], g[64:128, :])
            nc.scalar.dma_start(dst[64:128, PAD - 1:PAD + 511], g[0:64, :])
            nc.scalar.dma_start(dst[64:128, PAD + 511:PAD + 1023], g[64:128, :])

        def conv_ct(psp, cvtag, wtile, src):
            cv = psp.tile([128, 512], F32, tag=cvtag)
            for q in range(9):
                nc.tensor.matmul(cv[0:64, :], wtile[:, q, :],
                                 src[:, 2 * q:2 * q + 512],
                                 start=(q == 0), stop=(q == 8),
                                 tile_position=(0, 0))
                nc.tensor.matmul(cv[64:128, :], wtile[:, q, :],
                                 src[:, 2 * q + 512:2 * q + 1024],
                                 start=(q == 0), stop=(q == 8),
                                 tile_position=(0, 64))
            return cv

        def ph_conv1(b, psp, cvtag):
            cv1 = psp.tile([128, 512], F32, tag=cvtag)
            nc.tensor.matmul(cv1[0:64, :], w1t[:], a_sbs[b][:, 0:512],
                             start=True, stop=True, tile_position=(0, 0))
            nc.tensor.matmul(cv1[64:128, :], w1t[:], a_sbs[b][:, 512:1024],
                             start=True, stop=True, tile_position=(0, 64))
            g1 = ap.tile([128, 512], BF16, tag=f"g1{b}")
            nc.scalar.activation(g1[:], cv1[:], AF.Gelu, bias=b1c2[:])
            xb2 = ap.tile([128, PADB], BF16, tag=f"xb2{b}")
            asm_dual(xb2, g1)
            xb2s[b] = xb2

        def ph_conv2(b, psp, cvtag):
            cv2 = conv_ct(psp, cvtag, w2p, xb2s[b])
            x2b = ap.tile([128, 512], F32, tag=f"x2b{b}")
            stats = ap.tile([128, 2], F32, tag=f"stats{b}")
            sq = ap.tile([128, 512], F32, tag=f"sq{b}")
            nc.vector.tensor_scalar(
                out=x2b[:], in0=cv2[:], scalar1=b2c2[:], scalar2=0.0,
                op0=mybir.AluOpType.add, op1=mybir.AluOpType.add,
                accum_out=stats[:, 0:1])
            nc.vector.tensor_mul(sq[:], x2b[:], x2b[:])
            nc.vector.reduce_sum(stats[:, 1:2], sq[:], axis=mybir.AxisListType.X)
            x2bs[b] = x2b
            statss[b] = stats

        def ph_ln(b, psp, cvtag):
            stats = statss[b]
            x2b = x2bs[b]
            st16 = ap.tile([128, 2], BF16, tag=f"st16{b}")
            nc.vector.tensor_copy(st16[:], stats[:])
            totp = psp.tile([128, 64], F32, tag=cvtag)
            nc.tensor.matmul(totp[:, 0:2], ones128[:], st16[:], start=True, stop=True)

            mu = ap.tile([128, 1], F32, tag=f"mu{b}")
            msq = ap.tile([128, 1], F32, tag=f"msq{b}")
            var = ap.tile([128, 1], F32, tag=f"var{b}")
            sd = ap.tile([128, 1], F32, tag=f"sd{b}")
            rstd = ap.tile([128, 1], F32, tag=f"rstd{b}")
            nmr = ap.tile([128, 1], F32, tag=f"nmr{b}")
            inv_n = 1.0 / (64 * AUD)
            nc.vector.tensor_scalar_mul(mu[:], totp[:, 0:1], inv_n)
            nc.vector.tensor_scalar_mul(msq[:], totp[:, 1:2], inv_n)
            nc.vector.tensor_mul(var[:], mu[:], mu[:])
            nc.vector.tensor_sub(var[:], msq[:], var[:])
            nc.vector.tensor_scalar_add(var[:], var[:], EPS)
            nc.scalar.activation(sd[:], var[:], AF.Ln)
            nc.scalar.activation(rstd[:], sd[:], AF.Exp, scale=-0.5)
            nc.vector.tensor_mul(nmr[:], mu[:], rstd[:])
            nc.vector.tensor_scalar_mul(nmr[:], nmr[:], -1.0)

            t1 = ap.tile([128, 512], F32, tag=f"t1{b}")
            t2 = ap.tile([128, 512], F32, tag=f"t2{b}")
            g3 = ap.tile([128, 512], BF16, tag=f"g3{b}")
            nc.vector.tensor_scalar(out=t1[:], in0=x2b[:], scalar1=rstd[:],
                                    scalar2=nmr[:], op0=mybir.AluOpType.mult,
                                    op1=mybir.AluOpType.add)
            nc.vector.tensor_mul(t2[:], t1[:], lnw2[:])
            nc.vector.tensor_add(g3[:], t2[:], lnb2[:])
            xb3 = ap.tile([128, PADB], BF16, tag=f"xb3{b}")
            asm_dual(xb3, g3)
            xb3s[b] = xb3

        def ph_conv3(b, psp, cvtag):
            cv3 = conv_ct(psp, cvtag, w3p, xb3s[b])
            x_sb = ap.tile([128, 512], F32, tag=f"x_sb{b}")
            nc.vector.tensor_scalar(
                out=x_sb[:], in0=cv3[:], scalar1=b3c2[:], scalar2=0.0,
                op0=mybir.AluOpType.add, op1=mybir.AluOpType.add)
            x_sbs[b] = x_sb

        def ph_xt(b, psp, cvtag):
            xt = pp.tile([128, 8, 64], BF16, tag=f"xt{b}")
            for f in range(8):
                pt = psp.tile([128, 64], F32, tag=cvtag)
                if f < 4:
                    nc.tensor.transpose(pt[:], x_sbs[b][0:64, 128 * f:128 * f + 128],
                                        identA[0:64, :])
                else:
                    nc.tensor.transpose(pt[:],
                                        x_sbs[b][64:128, 128 * (f - 4):128 * (f - 4) + 128],
                                        identA[64:128, :], tile_position=(64, 0))
                nc.scalar.activation(xt[:, f, :], pt[:], AF.Copy)
            xts[b] = xt

        def ph_ktv(b, psp, cvtag):
            xt = xts[b]
            kt = pp.tile([128, 3, 64], BF16, tag=f"kt{b}")
            for m in range(3):
                ktp = psp.tile([128, 64], F32, tag=cvtag)
                for aj in range(8):
                    nc.tensor.matmul(ktp[:], wkt[:, aj, 128 * m:128 * m + 128],
                                     xt[:, aj, :], start=(aj == 0), stop=(aj == 7))
                nc.scalar.activation(kt[:, m, :], ktp[:], AF.Copy)

            v2p = psp.tile([128, 384], F32, tag=cvtag)
            for aj in range(8):
                nc.tensor.matmul(v2p[0:64, :], xt[:, aj, :], wvt[:, aj, :],
                                 start=(aj == 0), stop=(aj == 7),
                                 tile_position=(0, 0))
                nc.tensor.matmul(v2p[64:128, :], xt[:, aj, :], wvt[:, aj, :],
                                 start=(aj == 0), stop=(aj == 7),
                                 tile_position=(0, 64))
            v2 = pp.tile([128, 384], BF16, tag=f"v2{b}")
            nc.scalar.activation(v2[:], v2p[:], AF.Copy)

            # kp statics for sim (5 MMs/chunk)
            def mk_kp80(tag, ktsl):
                t = pp.tile([80, 128], BF16, tag=tag)
                nc.vector.tensor_mul(
                    t[:].rearrange("p (a j) -> p a j", a=2),
                    ktsl.broadcast_to([80, 2, 64]),
                    kmask_lo[:].rearrange("p (a j) -> p a j", a=2))
                return t

            kp0 = mk_kp80(f"kp0_{b}", kt[0:80, 0:1, :])
            kp2 = mk_kp80(f"kp2_{b}", kt[0:80, 1:2, :])
            kp3 = mk_kp80(f"kp3_{b}", kt[0:80, 2:3, :])
            kp1a = pp.tile([128, 128], BF16, tag=f"kp1a_{b}")
            nc.vector.tensor_mul(
                kp1a[64:128, :].rearrange("p (a j) -> p a j", a=2),
                kt[64:128, 0:1, :].broadcast_to([64, 2, 64]),
                kmask1a[64:128, :].rearrange("p (a j) -> p a j", a=2))
            kp1b = pp.tile([128, 128], BF16, tag=f"kp1b_{b}")
            nc.vector.tensor_mul(
                kp1b[64:128, :].rearrange("p (a j) -> p a j", a=2),
                kt[64:128, 1:2, :].broadcast_to([64, 2, 64]),
                kmask1b[64:128, :].rearrange("p (a j) -> p a j", a=2))
            kp_all[b] = (kp0, kp1a, kp1b, kp2, kp3)

            # vp value statics (baseline-style per (chunk, pair))
            vps = {}
            vp = pp.tile([128, 128], BF16, tag=f"vp00_{b}")     # (0, p0)
            nc.gpsimd.memset(vp[:, 80:128], 0.0)
            nc.vector.tensor_mul(vp[:, 0:80], v2[:, 0:80], vmask_v[:])
            vps[(0, 0)] = vp
            vp = pp.tile([128, 128], BF16, tag=f"vp01_{b}")     # (0, p1) dims 0:48
            nc.gpsimd.memset(vp[:, 0:80], 0.0)
            nc.vector.tensor_mul(vp[:, 80:128], v2[:, 80:128], vmask_v[:, 0:48])
            vps[(0, 1)] = vp
            vp = pp.tile([128, 112], BF16, tag=f"vp12_{b}")     # (1, p2)
            nc.gpsimd.memset(vp[:, 80:112], 0.0)
            nc.vector.tensor_mul(vp[:, 0:80], v2[:, 128:208], vmask_v[:])
            vps[(1, 2)] = vp
            vp = pp.tile([128, 112], BF16, tag=f"vp11_{b}")     # (1, p1) dims 48:80
            nc.gpsimd.memset(vp[:, 0:80], 0.0)
            nc.vector.tensor_mul(vp[:, 80:112], v2[:, 208:240], vmask_v[:, 48:80])
            vps[(1, 1)] = vp
            vp = pp.tile([128, 104], BF16, tag=f"vp23_{b}")     # (2, p3) + denoms
            nc.gpsimd.memset(vp[:, 80:104], 0.0)
            nc.vector.tensor_mul(vp[:, 0:80], v2[:, 256:336], vmask_v[:])
            nc.gpsimd.memset(vp[0:64, 102:103], 1.0)
            nc.gpsimd.memset(vp[64:128, 103:104], 1.0)
            vps[(2, 3)] = vp
            vp_all[b] = vps

        # denominator-only statics for at2 (batch-independent)
        vpd = []
        for p in range(3):
            t = cp.tile([128, 104], BF16, tag=f"vpd{p}")
            nc.gpsimd.memset(t[:], 0.0)
            nc.gpsimd.memset(t[0:64, 96 + 2 * p:97 + 2 * p], 1.0)
            nc.gpsimd.memset(t[64:128, 97 + 2 * p:98 + 2 * p], 1.0)
            vpd.append(t)

        # ---- pre-phase: conv1 both batches, then attention consts, audio(b0) ----
        ph_conv1(0, aps, "cv")
        ph_conv1(1, aps, "cv")

        b2c2 = cload("b2c2", (128, 1), F32)
        w3p = cload("w3p", (128, 9, 64), BF16)
        b3c2 = cload("b3c2", (128, 1), F32)
        lnw2 = cload("lnw2", (128, 512), F32)
        lnb2 = cload("lnb2", (128, 512), F32)
        identA = cload("identA", (128, 64), F32)
        wqt = cload("wqt", (128, 6, 384), BF16,
                    P["wqt"][:].rearrange("(n p) e -> p n e", p=128))
        cin_pre = {}
        for c0_ in range(4):
            t = cinp.tile([128, 6, TCH], BF16, tag="cin")
            nc.gpsimd.dma_start(
                t[:], P["ctx16"][0].rearrange("(n p) t -> p n t", p=128)
                [:, :, TCH * c0_:TCH * c0_ + TCH])
            cin_pre[(0, c0_)] = t
        wkt = cload("wkt", (128, 8, 384), BF16,
                    P["wkt"][:].rearrange("(n p) e -> p n e", p=128))
        wvt = cload("wvt", (128, 8, 384), BF16,
                    P["wvt"][:].rearrange("(n p) e -> p n e", p=128))
        woutA = cload("woutA", (128, 3, CTX), BF16,
                      P["woutA"][:].rearrange("(n p) c -> p n c", p=128))
        kmask_lo = cload("kmask_lo", (80, 128), BF16)
        kmask1a = cload("kmask1a", (128, 128), BF16)
        kmask1b = cload("kmask1b", (128, 128), BF16)
        vmask_v = cload("vmask_v", (128, 80), BF16)
        exp8 = cload("exp8", (104, 3, 128), BF16)

        def emit_q(cin, psum_pool, psum_tag):
            qt = qtp.tile([128, 3, TCH], BF16, tag="qt")
            for m in range(3):
                qp = psum_pool.tile([128, TCH], F32, tag=psum_tag)
                for n6 in range(6):
                    nc.tensor.matmul(qp[:], wqt[:, n6, 128 * m:128 * m + 128],
                                     cin[:, n6, :], start=(n6 == 0), stop=(n6 == 5))
                nc.vector.tensor_copy(qt[:, m, :], qp[:])
            return qt

        qt_pre = {}
        qt_pre[0] = emit_q(cin_pre[(0, 0)], aps, "qpre")
        ph_conv2(0, aps, "cv")
        qt_pre[1] = emit_q(cin_pre[(0, 1)], aps, "qpre")
        ph_ln(0, aps, "cv")
        qt_pre[2] = emit_q(cin_pre[(0, 2)], aps, "qpre")
        ph_conv3(0, aps, "cv")
        ph_xt(0, aps, "cv")
        ph_ktv(0, aps, "cv")
        qt_pre[3] = emit_q(cin_pre[(0, 3)], aps, "qpre")

        aps.release()

        # ---- main attention loop ----
        mps = tc.alloc_tile_pool(name="mps", bufs=2, space="PSUM")

        at_sbs = []
        for k2 in range(2):
            t = pp.tile([128, 3, TCH], BF16, tag=f"at_sb{k2}")
            nc.gpsimd.memset(t[64:96, 2, :], 0.0)
            nc.gpsimd.memset(t[96:97, 2, :], 1.0)
            at_sbs.append(t)

        pending_out = None

        def emit_out(job):
            ob, oc, oat = job
            for tt in range(4):
                of = ofp.tile([128, CTX], BF16, tag="of")
                for ci, (c0, cw) in enumerate(((0, 384), (384, 384))):
                    op = mps.tile([128, 512], F32, tag="ob")
                    for n in range(3):
                        nc.tensor.matmul(
                            op[:, 0:cw],
                            oat[0:RN[n], n, 128 * tt:128 * tt + 128],
                            woutA[0:RN[n], n, c0:c0 + cw],
                            start=(n == 0), stop=(n == 2))
                    if ci == 0:
                        nc.scalar.activation(of[:, c0:c0 + cw], op[:, 0:cw],
                                             AF.Copy)
                    else:
                        nc.vector.tensor_copy(of[:, c0:c0 + cw], op[:, 0:cw])
                nc.sync.dma_start(
                    out_e[ob, TCH * oc + 128 * tt: TCH * oc + 128 * tt + 128, :],
                    of[:])

        chunks = [(bb, cc2) for bb in range(BP) for cc2 in range(NCH)]
        ctx_aps = [P["ctx16"][bb].rearrange("(n p) t -> p n t", p=128)
                   for bb in range(BP)]
        cins = dict(cin_pre)
        qts = {i2: qt_pre[i2] for i2 in range(4)}

        b1_phases = {0: lambda: ph_conv2(1, mps, "qp"),
                     2: lambda: ph_ln(1, mps, "qp"),
                     4: lambda: ph_conv3(1, mps, "qp"),
                     5: lambda: ph_xt(1, mps, "qp"),
                     6: lambda: ph_ktv(1, mps, "qp")}

        for i, (b, c) in enumerate(chunks):
            kp0, kp1a, kp1b, kp2, kp3 = kp_all[b]
            vps = vp_all[b]
            for la in (3, 4):
                if i + la < len(chunks) and chunks[i + la] not in cins:
                    b3, c3 = chunks[i + la]
                    t = cinp.tile([128, 6, TCH], BF16, tag="cin")
                    nc.gpsimd.dma_start(t[:],
                                        ctx_aps[b3][:, :, TCH * c3:TCH * c3 + TCH])
                    cins[chunks[i + la]] = t

            qt = qts.pop(i)

            sim_defs = [
                [(kp0[:], qt[0:80, 0, :], None)],
                [(kp1a[64:128, :], qt[64:128, 0, :], (64, 0)),
                 (kp1b[64:128, :], qt[64:128, 1, :], (64, 0))],
                [(kp2[:], qt[0:80, 1, :], None)],
                [(kp3[:], qt[0:80, 2, :], None)],
            ]
            es = []
            for p in range(4):
                sp = mps.tile([128, TCH], F32, tag="sp")
                plan = sim_defs[p]
                for ii, (lh, rh, tp) in enumerate(plan):
                    nc.tensor.matmul(sp[:], lh, rh, start=(ii == 0),
                                     stop=(ii == len(plan) - 1),
                                     tile_position=tp)
                e = esp.tile([128, TCH], BF16, tag="es")
                nc.scalar.activation(e[:], sp[:], AF.Exp, scale=SCALE)
                es.append(e)

            # q for chunk i+2 fills the PE while exp runs on ACT
            if i + 2 < len(chunks) and (i + 2) not in qts:
                qts[i + 2] = emit_q(cins.pop(chunks[i + 2]), mps, "qp")

            at_sb = at_sbs[i % 2]

            # at2: chunk2 AV (pair3) + all denominators at rows 96:104
            at2 = mps.tile([104, TCH], F32, tag="at")
            at2_ops = [(vpd[0], 0), (vpd[1], 1), (vpd[2], 2), (vps[(2, 3)], 3)]
            for ii, (vpt, p) in enumerate(at2_ops):
                nc.tensor.matmul(at2[:], vpt[:, 0:104], es[p][:],
                                 start=(ii == 0), stop=(ii == 3))
            lnd = mp.tile([104, TCH], F32, tag="lnd")
            rec16 = mp.tile([104, TCH], BF16, tag="rec16")
            nc.scalar.activation(lnd[96:104, :], at2[96:104, :], AF.Ln)
            nc.scalar.activation(rec16[96:104, :], lnd[96:104, :], AF.Exp,
                                 scale=-1.0)

            def normalize(n, at_ps):
                brp = mps.tile([128, TCH], F32, tag="ob")
                nc.tensor.matmul(brp[0:ME[n], :], exp8[96:104, n, 0:ME[n]],
                                 rec16[96:104, :], start=True, stop=True,
                                 tile_position=(96, 0))
                brs = mp.tile([128, TCH], BF16, tag="brs")
                nc.vector.tensor_copy(brs[0:ME[n], :], brp[0:ME[n], :])
                nc.vector.tensor_mul(at_sb[0:ME[n], n, :],
                                     at_ps[0:ME[n], :], brs[0:ME[n], :])

            for n in (0, 1):
                a = mps.tile([128, TCH], F32, tag="at")
                W = 128 if n == 0 else 112
                prs = [(0, 0), (0, 1)] if n == 0 else [(1, 2), (1, 1)]
                for ii, key in enumerate(prs):
                    nc.tensor.matmul(a[0:W, :], vps[key][:], es[key[1]][:],
                                     start=(ii == 0), stop=(ii == 1))
                normalize(n, a)
            normalize(2, at2)

            if pending_out is not None:
                emit_out(pending_out)
            if i in b1_phases:
                b1_phases[i]()
            pending_out = (b, c, at_sb)

        emit_out(pending_out)

        mps.release()
        ap.release()
        ofp.release()
        mp.release()
        qtp.release()
        esp.release()
        cinp.release()
        pp.release()
        cp.release()

    split_waits(nc)
    return nc


def split_waits(nc, max_waits=1):
    """neuronxcc walrus accepts at most one attached sync wait per
    instruction; hoist extras onto standalone event-semaphore waits."""
    n_new = 0
    for f in nc.m.functions:
        for blk in f.blocks:
            new = []
            changed = False
            for inst in blk.instructions:
                si = inst.sync_info
                ow = list(si.on_wait) if (si is not None and si.on_wait) else []
                if len(ow) > max_waits:
                    for w in ow[:-max_waits]:
                        ev = mybir.InstEventSemaphore(
                            name=f"I-waitsplit-{n_new}", ins=[], outs=[])
                        ev.engine = inst.engine
                        ev.sync_info = mybir.SyncInfo(on_wait=[w], on_update=[])
                        nc.register_instruction(ev)
                        new.append(ev)
                        n_new += 1
                    inst.sync_info = mybir.SyncInfo(
                        on_wait=ow[-max_waits:], on_update=list(si.on_update))
                    changed = True
                new.append(inst)
            if changed:
                blk.instructions = new


_GRAPH = None


def kernel(**inputs):
    global _GRAPH
    if _GRAPH is None:
        _GRAPH = _build_graph()
    nc = _GRAPH

    inputs = {k: np.asarray(v, dtype=np.float32) for k, v in inputs.items()}
    consts = _build_host_consts(inputs)
    ctx = np.asarray(inputs["context"])
    audio = np.asarray(inputs["audio_context"])

    ctx16 = np.ascontiguousarray(ctx.transpose(0, 2, 1)).astype(BF)
    apad = np.zeros((B, AUD + 2 * PAD), np.float32)
    apad[:, PAD:PAD + AUD] = audio[:, 0, :]
    a_im = np.empty((B, KS, AUD), np.float32)
    for k in range(KS):
        a_im[:, k, :] = apad[:, k:k + AUD]
    a_im = a_im.astype(BF)

    in_maps = []
    for core in range(NCORES):
        m = dict(consts)
        s = slice(core * BP, (core + 1) * BP)
        m["ctx16"] = ctx16[s]
        m["a_im"] = a_im[s]
        in_maps.append(m)

    res = run_bass_kernel_spmd(nc, in_maps, list(range(NCORES)))
    out = np.concatenate([res.results[i]["out"] for i in range(NCORES)], axis=0)
    return out.astype(np.float32)
